# revision 26
# baseline (speedup 1.0000x reference)
"""Hex-masked sparse conv (ConvHex) as a Bass/Tile kernel on 8 TRN2 NeuronCores.

Strategy
--------
Data-parallel over batch: 16 images -> 2 per core.

The conv has 19 hex taps in a 9x5 window, C_in=64, C_out=128. All taps
have even dh+dw and the hex output mask is parity-sparse (only h+w even
survives), so the conv only ever touches the EVEN sub-lattice of x.
We pack x compactly on that lattice: row h keeps only columns w with
w = h%2 + 2k, giving a [64, 209*67] channel-major image in SBUF whose
row stride is 67 and whose tap offsets are stride-1 in k.

Partitions 0:64 hold x_c, partitions 64:128 hold x_c shifted by
134 slots (= 2 input rows). Taps that differ by (2,0) pair into K=128
full-array matmuls (8 pairs); the 3 leftover singles run as K=64
row-tiled matmuls on the two 64-row halves of the PE array (tile
positions (0,0)/(64,0)), staggered across groups so both halves run
concurrently on different PSUM banks. Effective cost: 9.5 array passes
per output group instead of 11.

Each matmul computes a group of up to 7 same-parity output rows
(free AP [rows, cols], steps [134, 1]) accumulating into one PSUM bank.
A short burst of dummy matmuls at t=0 keeps the PE busy while the first
input chunks stream in, so the HAM clock gate reaches 2.4 GHz before
real work starts. Epilogue: elu(z)+1 = min(exp(z),1) + relu(z) via one
ScalarE exp + relu and a VectorE merge, written as bf16 to a compact
channel-major output [201, 128, 65] that the host scatters back to NHWC.
"""

import numpy as np
import ml_dtypes

# ---------------------------------------------------------------- constants
R = 2
CIN, COUT = 64, 128
H, W = 209, 133
OH, OW = H - 4 * R, W - 2 * R   # 201, 129
NK = 67                         # compact slots per input row
XLEN = H * NK                   # 14003
SHIFT = 2 * NK                  # 134 slots = 2 input rows
NBATCH, NCORES = 16, 8
NPER = NBATCH // NCORES         # 2 images per core
NROWS = 7                       # output rows per matmul group
SLOTS = 65                      # max stride-2 columns per output row
PAD = 160                       # sbuf free-dim padding so row-slab APs stay in bounds
NWARM = 28                      # dummy matmuls to pre-warm the PE clock gate
NCHUNK = 4                      # input DMA chunks per image half

BF16 = ml_dtypes.bfloat16


def _hex_indices(radius):
    moves = np.array([[1, 1], [2, 0], [1, -1], [-1, -1], [-2, 0], [-1, 1]])
    out = [[2 * radius, radius]]
    for il in range(1, radius + 1):
        s = np.array([[2 * radius - 2 * il, radius]])
        cur = moves.repeat(il, axis=0).cumsum(axis=0)
        out.extend((s + cur).tolist())
    return np.array(out, dtype=np.int32)


def _make_out_mask():
    mr = (OW - 1) // 2
    f = np.zeros((mr * 4 + 1, mr * 2 + 1), dtype=np.float32)
    for ind in _hex_indices(mr):
        f[tuple(ind)] = 1.0
    i_cut = (mr * 4 + 1 - OH) // 2
    return f[i_cut:-i_cut, :]    # [OH, OW]


_TAPS = _hex_indices(R)          # [19, 2] (dh, dw), reference tap order j
_NTAPS = len(_TAPS)
_MASK = _make_out_mask()         # [201, 129] float32


def _tap_off(tap, p):
    """Flat compact-lattice offset of tap (dh, dw) for output parity p."""
    dh, dw = int(tap[0]), int(tap[1])
    if dh % 2 == 0:
        return NK * dh + dw // 2
    return NK * dh + (dw - 1) // 2 + p


def _make_streams():
    """Pair taps along (2,0). Returns (pairs, singles) as tap indices."""
    idx = {tuple(t): j for j, t in enumerate(_TAPS.tolist())}
    used = set()
    pairs, singles = [], []
    for t in sorted(idx):
        if t in used or (t[0] - 2, t[1]) in idx:
            continue
        chain = [t]
        cur = t
        while (cur[0] + 2, cur[1]) in idx:
            cur = (cur[0] + 2, cur[1])
            chain.append(cur)
        for k in range(0, len(chain) - 1, 2):
            pairs.append((idx[chain[k]], idx[chain[k + 1]]))
            used.update(chain[k:k + 2])
        if len(chain) % 2:
            singles.append(idx[chain[-1]])
            used.add(chain[-1])
    assert len(pairs) * 2 + len(singles) == _NTAPS
    return pairs, singles


_PAIRS, _SINGLES = _make_streams()   # 8 pairs + 3 singles


def _make_groups():
    """Groups of <=NROWS same-parity output rows sharing one PSUM bank.

    Returns list of (h0, nrows, k0, ncols, p): rows h0, h0+2, ...,
    h0+2*(nrows-1); slots k0..k0+ncols-1 (slot k of row h <-> w = p + 2k).
    """
    spans = []
    for h in range(OH):
        w_act = np.nonzero(_MASK[h])[0]
        spans.append((int(w_act[0]), int(w_act[-1])))
    groups = []
    for p in (0, 1):
        rows = list(range(p, OH, 2))
        for i in range(0, len(rows), NROWS):
            chunk = rows[i:i + NROWS]
            w_lo = min(spans[h][0] for h in chunk)
            w_hi = max(spans[h][1] for h in chunk)
            groups.append((chunk[0], len(chunk), (w_lo - p) // 2,
                           (w_hi - w_lo) // 2 + 1, p))
    groups.sort(key=lambda g: g[0])
    return groups


_GROUPS = _make_groups()


# block sizes: 6-group blocks leave 2 spare PSUM banks (of 8) so the next
# block's matmuls never wait on trailing epilogues; the tapered tail keeps
# the final blocks small so their epilogues overlap earlier blocks' matmuls
def _make_blocks():
    sizes = [6] * ((len(_GROUPS) - 6) // 6) + [4, 2]
    if sum(sizes) != len(_GROUPS):
        sizes = [7] * (len(_GROUPS) // 7) + [len(_GROUPS) % 7]
    blocks = []
    i = 0
    for s in sizes:
        blocks.append(list(range(i, i + s)))
        i += s
    return blocks


_BLOCKS = _make_blocks()


def _make_slabs():
    """Output row-slabs per group: after group gi (in _GROUPS order), rows
    [a, b) of the compact output are fully written and can DMA to DRAM.

    Returns list (per group) of (h_lo, h_hi) with h_hi exclusive; empty
    ranges mean no new complete rows after that group.
    """
    owner = {}
    for gi, (h0, nrows, k0, ncols, p) in enumerate(_GROUPS):
        for r in range(nrows):
            owner[h0 + 2 * r] = gi
    slabs = []
    prev = 0
    for gi in range(len(_GROUPS)):
        h = prev
        while h < OH and owner[h] <= gi:
            h += 1
        slabs.append((prev, h))
        prev = h
    assert prev == OH
    return slabs


_SLABS = _make_slabs()


def _assign_tiles(block):
    """Greedy nf-balanced assignment of a block's singles to PE halves.

    Returns half[i] in {0, 1} (0 -> rows 0:64 / tile (0,0), 1 -> rows
    64:128 / tile (64,0)) such that concurrent singles land on different
    PSUM banks with near-equal column load per half.
    """
    order = sorted(range(len(block)), key=lambda i: -block[i][1] * block[i][3])
    load = [0, 0]
    half = [0] * len(block)
    for i in order:
        h = 0 if load[0] <= load[1] else 1
        half[i] = h
        load[h] += block[i][1] * block[i][3]
    return half


# ---------------------------------------------------------------- device program
_PROGRAM = None


def _build_program():
    import concourse.mybir as mybir
    from concourse import bacc
    from concourse.tile import TileContext

    f32 = mybir.dt.float32
    bf16 = mybir.dt.bfloat16
    Alu = mybir.AluOpType
    Act = mybir.ActivationFunctionType

    # Bacc (not plain Bass): its compile() legalizes sync waits for the
    # TRN2 1-wait-per-instruction limit via generate_event_semaphores
    nc = bacc.Bacc("TRN2", target_bir_lowering=False, debug=False)
    xt_in = nc.declare_dram_parameter("xt", [NPER, CIN, XLEN], bf16, isOutput=False)
    wp_in = nc.declare_dram_parameter("wp", [128, len(_PAIRS) * COUT], bf16, isOutput=False)
    ws_in = nc.declare_dram_parameter("ws", [128, len(_SINGLES) * COUT], bf16, isOutput=False)
    bias_in = nc.declare_dram_parameter("bias", [COUT, 1], f32, isOutput=False)
    out_p = nc.declare_dram_parameter("out", [NPER, COUT, OH, SLOTS], bf16, isOutput=True)

    with TileContext(nc) as tc:
        with (
            tc.tile_pool(name="const", bufs=1) as cpool,
            tc.tile_pool(name="x", bufs=2) as xpool,
            tc.tile_pool(name="ps", bufs=8, space="PSUM") as pspool,
            tc.tile_pool(name="ep", bufs=4) as epool,
            tc.tile_pool(name="rp", bufs=4) as rpool,
            tc.tile_pool(name="sp", bufs=2) as spool,
        ):
            # weights prepacked host-side into the exact SBUF layout: one
            # DMA each with full-partition-line packets
            # both images' input tiles up front; the first chunk pair is
            # triggered before anything else so the first block's deps land
            # as early as possible, and image 1's input is never queued
            # behind image 0's output slabs (Sync-queue head-of-line)
            xt_ts = [xpool.tile([128, XLEN + PAD], bf16, name=f"xt{n}")
                     for n in range(NPER)]
            cb = [XLEN * c // NCHUNK for c in range(NCHUNK + 1)]
            ub = [min(b, XLEN - SHIFT) for b in cb]
            nc.sync.dma_start(xt_ts[0][0:CIN, cb[0]:cb[1]],
                              xt_in[0, :, cb[0]:cb[1]])
            nc.sync.dma_start(xt_ts[0][CIN:128, ub[0]:ub[1]],
                              xt_in[0, :, ub[0] + SHIFT:ub[1] + SHIFT])

            wp_t = cpool.tile([128, len(_PAIRS) * COUT], bf16)
            nc.sync.dma_start(wp_t[:], wp_in[:])
            ws_t = cpool.tile([128, len(_SINGLES) * COUT], bf16)
            nc.sync.dma_start(ws_t[:], ws_in[:])
            bias_t = cpool.tile([COUT, 1], f32)
            nc.sync.dma_start(bias_t[:], bias_in[:])
            # warmup activations: preload the ACT function tables and absorb
            # the bias-DMA wait so no steady-state ACT needs >2 sync waits
            warm_t = cpool.tile([1, 1], f32)
            nc.scalar.activation(warm_t[0:1, 0:1], bias_t[0:1, 0:1], Act.Exp)
            nc.scalar.activation(warm_t[0:1, 0:1], bias_t[0:1, 0:1], Act.Relu)

            # PE clock-gate warmup: a burst of accumulating dummy matmuls
            # that depends only on an on-chip memset, so it runs during the
            # first input DMA and un-throttles the HAM before real matmuls
            wm_s = cpool.tile([128, 256], bf16)
            nc.vector.memset(wm_s[:], 0.0)
            wm_ps = pspool.tile([128, 512], f32, name="wps", tag="psb")
            for i in range(NWARM):
                nc.tensor.matmul(wm_ps[:, 0:256], wm_s[:, 0:128], wm_s[:, 0:256],
                                 start=(i == 0), stop=(i == NWARM - 1))

            # remaining input chunks, lower/upper interleaved per chunk
            for n in range(NPER):
                for c in range(NCHUNK):
                    if n == 0 and c == 0:
                        continue
                    nc.sync.dma_start(xt_ts[n][0:CIN, cb[c]:cb[c + 1]],
                                      xt_in[n, :, cb[c]:cb[c + 1]])
                    if ub[c] < ub[c + 1]:
                        nc.sync.dma_start(
                            xt_ts[n][CIN:128, ub[c]:ub[c + 1]],
                            xt_in[n, :, ub[c] + SHIFT:ub[c + 1] + SHIFT])

            for n in range(NPER):
                xt_t = xt_ts[n]
                # whole-image compact output staged in SBUF (65-slot rows);
                # DMA'd to DRAM in big contiguous row-slabs so the DMA
                # engines move multi-KB runs per channel instead of 130B
                bs_t = spool.tile([128, (OH + 1) * SLOTS], bf16,
                                  name="bst", tag="bst")

                def rhs_ap(h0, nrows, k0, ncols, p, tap, base, kpart):
                    o0 = (h0 * NK + k0 + _tap_off(_TAPS[tap], p)
                          - (SHIFT if base else 0))
                    sl = xt_t[base:base + kpart, o0:o0 + SHIFT * nrows]
                    return sl.rearrange("q (h w) -> q h w", h=nrows)[:, :, 0:ncols]

                for bidx in _BLOCKS:
                    block = [_GROUPS[i] for i in bidx]
                    halves = _assign_tiles(block)
                    tiles = [pspool.tile([128, 512], f32, name="psb", tag="psb")
                             for _ in block]

                    def pv(g):
                        h0, nrows, k0, ncols, p = block[g]
                        return tiles[g][:, 0:nrows * ncols].rearrange(
                            "q (h w) -> q h w", h=nrows)

                    # 8 paired taps: K=128 full-array matmuls
                    for s, (lo, _hi) in enumerate(_PAIRS):
                        lhsT = wp_t[:, s * COUT:(s + 1) * COUT]
                        for g, (h0, nrows, k0, ncols, p) in enumerate(block):
                            nc.tensor.matmul(
                                pv(g), lhsT,
                                rhs_ap(h0, nrows, k0, ncols, p, lo, 0, 128),
                                start=(s == 0), stop=False)
                    # 3 singles: K=64 row-tiled, staggered across PE halves
                    for si, j in enumerate(_SINGLES):
                        for g, (h0, nrows, k0, ncols, p) in enumerate(block):
                            base = 64 * halves[g]
                            lhsT = ws_t[base:base + CIN,
                                        si * COUT:(si + 1) * COUT]
                            nc.tensor.matmul(
                                pv(g), lhsT,
                                rhs_ap(h0, nrows, k0, ncols, p, j, base, CIN),
                                start=False, stop=(si == len(_SINGLES) - 1))

                    for g, (h0, nrows, k0, ncols, p) in enumerate(block):
                        nf = nrows * ncols
                        pvg = tiles[g][:, 0:nf]
                        e_t = epool.tile([128, 512], bf16)
                        a_t = rpool.tile([128, 512], bf16, name="at", tag="at")
                        # device computes elu(z)+1 = min(exp(z),1) + relu(z),
                        # z = conv + bias; the host subtracts the 1 during
                        # reassembly. ACT is the only PSUM reader and DVE the
                        # only pre-DMA writer, keeping every instruction
                        # within its ISA sync-wait slot budget.
                        nc.scalar.activation(e_t[:, 0:nf], pvg, Act.Exp,
                                             bias=bias_t[:, 0:1])
                        nc.scalar.activation(a_t[:, 0:nf], pvg, Act.Relu,
                                             bias=bias_t[:, 0:1])
                        dst = bs_t[:, h0 * SLOTS:(h0 + 2 * nrows) * SLOTS]
                        dst = dst.rearrange("q (h w) -> q h w", h=nrows)
                        nc.vector.scalar_tensor_tensor(
                            dst[:, :, k0:k0 + ncols],
                            e_t[:, 0:nf].rearrange("q (h w) -> q h w", h=nrows),
                            1.0,
                            a_t[:, 0:nf].rearrange("q (h w) -> q h w", h=nrows),
                            op0=Alu.min, op1=Alu.add)
                        h_lo, h_hi = _SLABS[bidx[g]]
                        if h_hi > h_lo:
                            nc.sync.dma_start(
                                out_p[n, :, h_lo:h_hi, :],
                                bs_t[:, h_lo * SLOTS:h_hi * SLOTS].rearrange(
                                    "q (h w) -> q h w", h=h_hi - h_lo))
    nc.compile()
    return nc


def _get_program():
    global _PROGRAM
    if _PROGRAM is None:
        _PROGRAM = _build_program()
    return _PROGRAM


# ---------------------------------------------------------------- host wrapper
def _prep_inputs(x, sparse_weights, offset):
    x = np.asarray(x, np.float32)
    xe = np.zeros((NBATCH, H, NK, CIN), np.float32)
    xe[:, 0::2, :67] = x[:, 0::2, 0::2]
    xe[:, 1::2, :66] = x[:, 1::2, 1::2]
    xt = np.ascontiguousarray(xe.transpose(0, 3, 1, 2)).reshape(
        NBATCH, CIN, XLEN).astype(BF16)
    sw3 = np.asarray(sparse_weights, np.float32).reshape(CIN, COUT, _NTAPS)
    # prepacked flat [128, n*COUT] in the exact SBUF layout
    wp = np.empty((128, len(_PAIRS) * COUT), np.float32)
    for s, (lo, hi) in enumerate(_PAIRS):
        wp[0:CIN, s * COUT:(s + 1) * COUT] = sw3[:, :, lo]
        wp[CIN:128, s * COUT:(s + 1) * COUT] = sw3[:, :, hi]
    # singles duplicated into both partition halves so either PE row-tile
    # can load them
    ws = np.empty((128, len(_SINGLES) * COUT), np.float32)
    for s, j in enumerate(_SINGLES):
        ws[0:CIN, s * COUT:(s + 1) * COUT] = sw3[:, :, j]
        ws[CIN:128, s * COUT:(s + 1) * COUT] = sw3[:, :, j]
    bias = np.asarray(offset, np.float32).reshape(COUT, 1)
    return xt, wp.astype(BF16), ws.astype(BF16), bias


def _make_in_maps(inputs):
    xt, wp, ws, bias = _prep_inputs(**inputs)
    return [
        {"xt": xt[c * NPER:(c + 1) * NPER], "wp": wp, "ws": ws, "bias": bias}
        for c in range(NCORES)
    ]


def kernel(x, sparse_weights, offset):
    from concourse.bass_utils import run_bass_kernel_spmd

    nc = _get_program()
    in_maps = _make_in_maps(
        {"x": x, "sparse_weights": sparse_weights, "offset": offset})
    res = run_bass_kernel_spmd(nc, in_maps, list(range(NCORES)))
    arr = np.concatenate([np.asarray(res.results[c]["out"]).astype(np.float32)
                          for c in range(NCORES)], axis=0)  # [16, 128, 201, 65]
    arr -= 1.0   # device returns elu(z) + 1
    arr = arr.transpose(0, 2, 3, 1)                         # [16, 201, 65, 128]
    full = np.zeros((NBATCH, OH, OW, COUT), np.float32)
    full[:, 0::2, 0::2, :] = arr[:, 0::2]
    full[:, 1::2, 1::2, :] = arr[:, 1::2, :64]
    full[:, _MASK == 0] = 0.0
    return full


# revision 27
# speedup vs baseline: 1.1324x; 1.1324x over previous
"""Hex-masked sparse conv (ConvHex) as a Bass/Tile kernel on 8 TRN2 NeuronCores.

Strategy
--------
Data-parallel over batch: 16 images -> 2 per core.

The conv has 19 hex taps in a 9x5 window, C_in=64, C_out=128. All taps
have even dh+dw and the hex output mask is parity-sparse (only h+w even
survives), so the conv only ever touches the EVEN sub-lattice of x.
We pack x compactly on that lattice: row h keeps only columns w with
w = h%2 + 2k, giving a [64, 209*67] channel-major image in SBUF whose
row stride is 67 and whose tap offsets are stride-1 in k.

Partitions 0:64 hold x_c, partitions 64:128 hold x_c shifted by
134 slots (= 2 input rows). Taps that differ by (2,0) pair into K=128
full-array matmuls (8 pairs); the 3 leftover singles run as K=64
row-tiled matmuls on the two 64-row halves of the PE array (tile
positions (0,0)/(64,0)), staggered across groups so both halves run
concurrently on different PSUM banks. Effective cost: 9.5 array passes
per output group instead of 11.

Each matmul computes a group of up to 7 same-parity output rows
(free AP [rows, cols], steps [134, 1]) accumulating into one PSUM bank.
A short burst of dummy matmuls at t=0 keeps the PE busy while the first
input chunks stream in, so the HAM clock gate reaches 2.4 GHz before
real work starts. Epilogue: elu(z)+1 = min(exp(z),1) + relu(z) via one
ScalarE exp + relu and a VectorE merge, written as bf16 to a compact
channel-major output [201, 128, 65] that the host scatters back to NHWC.
"""

import numpy as np
import ml_dtypes

# ---------------------------------------------------------------- constants
R = 2
CIN, COUT = 64, 128
H, W = 209, 133
OH, OW = H - 4 * R, W - 2 * R   # 201, 129
NK = 67                         # compact slots per input row
XLEN = H * NK                   # 14003
SHIFT = 2 * NK                  # 134 slots = 2 input rows
NBATCH, NCORES = 16, 8
NPER = NBATCH // NCORES         # 2 images per core
NROWS = 7                       # output rows per matmul group
SLOTS = 65                      # max stride-2 columns per output row
PAD = 160                       # sbuf free-dim padding so row-slab APs stay in bounds
NWARM = 28                      # dummy matmuls to pre-warm the PE clock gate
NCHUNK = 4                      # input DMA chunks per image half

BF16 = ml_dtypes.bfloat16


def _hex_indices(radius):
    moves = np.array([[1, 1], [2, 0], [1, -1], [-1, -1], [-2, 0], [-1, 1]])
    out = [[2 * radius, radius]]
    for il in range(1, radius + 1):
        s = np.array([[2 * radius - 2 * il, radius]])
        cur = moves.repeat(il, axis=0).cumsum(axis=0)
        out.extend((s + cur).tolist())
    return np.array(out, dtype=np.int32)


def _make_out_mask():
    mr = (OW - 1) // 2
    f = np.zeros((mr * 4 + 1, mr * 2 + 1), dtype=np.float32)
    for ind in _hex_indices(mr):
        f[tuple(ind)] = 1.0
    i_cut = (mr * 4 + 1 - OH) // 2
    return f[i_cut:-i_cut, :]    # [OH, OW]


_TAPS = _hex_indices(R)          # [19, 2] (dh, dw), reference tap order j
_NTAPS = len(_TAPS)
_MASK = _make_out_mask()         # [201, 129] float32


def _tap_off(tap, p):
    """Flat compact-lattice offset of tap (dh, dw) for output parity p."""
    dh, dw = int(tap[0]), int(tap[1])
    if dh % 2 == 0:
        return NK * dh + dw // 2
    return NK * dh + (dw - 1) // 2 + p


def _make_streams():
    """Pair taps along (2,0). Returns (pairs, singles) as tap indices."""
    idx = {tuple(t): j for j, t in enumerate(_TAPS.tolist())}
    used = set()
    pairs, singles = [], []
    for t in sorted(idx):
        if t in used or (t[0] - 2, t[1]) in idx:
            continue
        chain = [t]
        cur = t
        while (cur[0] + 2, cur[1]) in idx:
            cur = (cur[0] + 2, cur[1])
            chain.append(cur)
        for k in range(0, len(chain) - 1, 2):
            pairs.append((idx[chain[k]], idx[chain[k + 1]]))
            used.update(chain[k:k + 2])
        if len(chain) % 2:
            singles.append(idx[chain[-1]])
            used.add(chain[-1])
    assert len(pairs) * 2 + len(singles) == _NTAPS
    return pairs, singles


_PAIRS, _SINGLES = _make_streams()   # 8 pairs + 3 singles


def _make_groups():
    """Groups of <=NROWS same-parity output rows sharing one PSUM bank.

    Returns list of (h0, nrows, k0, ncols, p): rows h0, h0+2, ...,
    h0+2*(nrows-1); slots k0..k0+ncols-1 (slot k of row h <-> w = p + 2k).
    """
    spans = []
    for h in range(OH):
        w_act = np.nonzero(_MASK[h])[0]
        spans.append((int(w_act[0]), int(w_act[-1])))
    groups = []
    for p in (0, 1):
        rows = list(range(p, OH, 2))
        for i in range(0, len(rows), NROWS):
            chunk = rows[i:i + NROWS]
            w_lo = min(spans[h][0] for h in chunk)
            w_hi = max(spans[h][1] for h in chunk)
            groups.append((chunk[0], len(chunk), (w_lo - p) // 2,
                           (w_hi - w_lo) // 2 + 1, p))
    groups.sort(key=lambda g: g[0])
    return groups


_GROUPS = _make_groups()


# 7-group blocks: exactly one spare PSUM bank. With 2+ spares the scheduler
# interleaves the next block's full-array pairs into this block's row-tiled
# singles phase and the PE thrashes tiling modes (measured +20% matmul time);
# with one spare the phases stay cleanly separated
def _make_blocks():
    sizes = [7] * (len(_GROUPS) // 7)
    if len(_GROUPS) % 7:
        sizes.append(len(_GROUPS) % 7)
    blocks = []
    i = 0
    for s in sizes:
        blocks.append(list(range(i, i + s)))
        i += s
    return blocks


_BLOCKS = _make_blocks()


def _make_slabs():
    """Output row-slabs per group: after group gi (in _GROUPS order), rows
    [a, b) of the compact output are fully written and can DMA to DRAM.

    Returns list (per group) of (h_lo, h_hi) with h_hi exclusive; empty
    ranges mean no new complete rows after that group.
    """
    owner = {}
    for gi, (h0, nrows, k0, ncols, p) in enumerate(_GROUPS):
        for r in range(nrows):
            owner[h0 + 2 * r] = gi
    slabs = []
    prev = 0
    for gi in range(len(_GROUPS)):
        h = prev
        while h < OH and owner[h] <= gi:
            h += 1
        slabs.append((prev, h))
        prev = h
    assert prev == OH
    return slabs


_SLABS = _make_slabs()


def _assign_tiles(block):
    """Greedy nf-balanced assignment of a block's singles to PE halves.

    Returns half[i] in {0, 1} (0 -> rows 0:64 / tile (0,0), 1 -> rows
    64:128 / tile (64,0)) such that concurrent singles land on different
    PSUM banks with near-equal column load per half.
    """
    order = sorted(range(len(block)), key=lambda i: -block[i][1] * block[i][3])
    load = [0, 0]
    half = [0] * len(block)
    for i in order:
        h = 0 if load[0] <= load[1] else 1
        half[i] = h
        load[h] += block[i][1] * block[i][3]
    return half


# ---------------------------------------------------------------- device program
_PROGRAM = None


def _build_program():
    import concourse.mybir as mybir
    from concourse import bacc
    from concourse.tile import TileContext

    f32 = mybir.dt.float32
    bf16 = mybir.dt.bfloat16
    Alu = mybir.AluOpType
    Act = mybir.ActivationFunctionType

    # Bacc (not plain Bass): its compile() legalizes sync waits for the
    # TRN2 1-wait-per-instruction limit via generate_event_semaphores
    nc = bacc.Bacc("TRN2", target_bir_lowering=False, debug=False)
    xt_in = nc.declare_dram_parameter("xt", [NPER, CIN, XLEN], bf16, isOutput=False)
    wp_in = nc.declare_dram_parameter("wp", [128, len(_PAIRS) * COUT], bf16, isOutput=False)
    ws_in = nc.declare_dram_parameter("ws", [128, len(_SINGLES) * COUT], bf16, isOutput=False)
    bias_in = nc.declare_dram_parameter("bias", [COUT, 1], f32, isOutput=False)
    out_p = nc.declare_dram_parameter("out", [NPER, COUT, OH, SLOTS], bf16, isOutput=True)

    with TileContext(nc) as tc:
        with (
            tc.tile_pool(name="const", bufs=1) as cpool,
            tc.tile_pool(name="x", bufs=2) as xpool,
            tc.tile_pool(name="ps", bufs=8, space="PSUM") as pspool,
            tc.tile_pool(name="ep", bufs=4) as epool,
            tc.tile_pool(name="rp", bufs=4) as rpool,
            tc.tile_pool(name="sp", bufs=2) as spool,
        ):
            # weights prepacked host-side into the exact SBUF layout: one
            # DMA each with full-partition-line packets
            # both images' input tiles up front; the first chunk pair is
            # triggered before anything else so the first block's deps land
            # as early as possible, and image 1's input is never queued
            # behind image 0's output slabs (Sync-queue head-of-line)
            xt_ts = [xpool.tile([128, XLEN + PAD], bf16, name=f"xt{n}")
                     for n in range(NPER)]
            cb = [XLEN * c // NCHUNK for c in range(NCHUNK + 1)]
            ub = [min(b, XLEN - SHIFT) for b in cb]
            nc.sync.dma_start(xt_ts[0][0:CIN, cb[0]:cb[1]],
                              xt_in[0, :, cb[0]:cb[1]])
            nc.sync.dma_start(xt_ts[0][CIN:128, ub[0]:ub[1]],
                              xt_in[0, :, ub[0] + SHIFT:ub[1] + SHIFT])

            wp_t = cpool.tile([128, len(_PAIRS) * COUT], bf16)
            nc.sync.dma_start(wp_t[:], wp_in[:])
            ws_t = cpool.tile([128, len(_SINGLES) * COUT], bf16)
            nc.sync.dma_start(ws_t[:], ws_in[:])
            bias_t = cpool.tile([COUT, 1], f32)
            nc.sync.dma_start(bias_t[:], bias_in[:])
            # warmup activations: preload the ACT function tables and absorb
            # the bias-DMA wait so no steady-state ACT needs >2 sync waits
            warm_t = cpool.tile([1, 1], f32)
            nc.scalar.activation(warm_t[0:1, 0:1], bias_t[0:1, 0:1], Act.Exp)
            nc.scalar.activation(warm_t[0:1, 0:1], bias_t[0:1, 0:1], Act.Relu)

            # PE clock-gate warmup: a burst of accumulating dummy matmuls
            # that depends only on an on-chip memset, so it runs during the
            # first input DMA and un-throttles the HAM before real matmuls
            wm_s = cpool.tile([128, 256], bf16)
            nc.vector.memset(wm_s[:], 0.0)
            wm_ps = pspool.tile([128, 512], f32, name="wps", tag="psb")
            for i in range(NWARM):
                nc.tensor.matmul(wm_ps[:, 0:256], wm_s[:, 0:128], wm_s[:, 0:256],
                                 start=(i == 0), stop=(i == NWARM - 1))

            # remaining input chunks, lower/upper interleaved per chunk
            for n in range(NPER):
                for c in range(NCHUNK):
                    if n == 0 and c == 0:
                        continue
                    nc.sync.dma_start(xt_ts[n][0:CIN, cb[c]:cb[c + 1]],
                                      xt_in[n, :, cb[c]:cb[c + 1]])
                    if ub[c] < ub[c + 1]:
                        nc.sync.dma_start(
                            xt_ts[n][CIN:128, ub[c]:ub[c + 1]],
                            xt_in[n, :, ub[c] + SHIFT:ub[c + 1] + SHIFT])

            for n in range(NPER):
                xt_t = xt_ts[n]
                # whole-image compact output staged in SBUF (65-slot rows);
                # DMA'd to DRAM in big contiguous row-slabs so the DMA
                # engines move multi-KB runs per channel instead of 130B
                bs_t = spool.tile([128, (OH + 1) * SLOTS], bf16,
                                  name="bst", tag="bst")

                def rhs_ap(h0, nrows, k0, ncols, p, tap, base, kpart):
                    o0 = (h0 * NK + k0 + _tap_off(_TAPS[tap], p)
                          - (SHIFT if base else 0))
                    sl = xt_t[base:base + kpart, o0:o0 + SHIFT * nrows]
                    return sl.rearrange("q (h w) -> q h w", h=nrows)[:, :, 0:ncols]

                for bidx in _BLOCKS:
                    block = [_GROUPS[i] for i in bidx]
                    halves = _assign_tiles(block)
                    tiles = [pspool.tile([128, 512], f32, name="psb", tag="psb")
                             for _ in block]

                    def pv(g):
                        h0, nrows, k0, ncols, p = block[g]
                        return tiles[g][:, 0:nrows * ncols].rearrange(
                            "q (h w) -> q h w", h=nrows)

                    # 8 paired taps: K=128 full-array matmuls
                    for s, (lo, _hi) in enumerate(_PAIRS):
                        lhsT = wp_t[:, s * COUT:(s + 1) * COUT]
                        for g, (h0, nrows, k0, ncols, p) in enumerate(block):
                            nc.tensor.matmul(
                                pv(g), lhsT,
                                rhs_ap(h0, nrows, k0, ncols, p, lo, 0, 128),
                                start=(s == 0), stop=False)
                    # 3 singles: K=64 row-tiled, staggered across PE halves
                    for si, j in enumerate(_SINGLES):
                        for g, (h0, nrows, k0, ncols, p) in enumerate(block):
                            base = 64 * halves[g]
                            lhsT = ws_t[base:base + CIN,
                                        si * COUT:(si + 1) * COUT]
                            nc.tensor.matmul(
                                pv(g), lhsT,
                                rhs_ap(h0, nrows, k0, ncols, p, j, base, CIN),
                                start=False, stop=(si == len(_SINGLES) - 1))

                    for g, (h0, nrows, k0, ncols, p) in enumerate(block):
                        nf = nrows * ncols
                        pvg = tiles[g][:, 0:nf]
                        e_t = epool.tile([128, 512], bf16)
                        a_t = rpool.tile([128, 512], bf16, name="at", tag="at")
                        # device computes elu(z)+1 = min(exp(z),1) + relu(z),
                        # z = conv + bias; the host subtracts the 1 during
                        # reassembly. ACT is the only PSUM reader and DVE the
                        # only pre-DMA writer, keeping every instruction
                        # within its ISA sync-wait slot budget.
                        nc.scalar.activation(e_t[:, 0:nf], pvg, Act.Exp,
                                             bias=bias_t[:, 0:1])
                        nc.scalar.activation(a_t[:, 0:nf], pvg, Act.Relu,
                                             bias=bias_t[:, 0:1])
                        dst = bs_t[:, h0 * SLOTS:(h0 + 2 * nrows) * SLOTS]
                        dst = dst.rearrange("q (h w) -> q h w", h=nrows)
                        nc.vector.scalar_tensor_tensor(
                            dst[:, :, k0:k0 + ncols],
                            e_t[:, 0:nf].rearrange("q (h w) -> q h w", h=nrows),
                            1.0,
                            a_t[:, 0:nf].rearrange("q (h w) -> q h w", h=nrows),
                            op0=Alu.min, op1=Alu.add)
                        h_lo, h_hi = _SLABS[bidx[g]]
                        if h_hi > h_lo:
                            nc.sync.dma_start(
                                out_p[n, :, h_lo:h_hi, :],
                                bs_t[:, h_lo * SLOTS:h_hi * SLOTS].rearrange(
                                    "q (h w) -> q h w", h=h_hi - h_lo))
    nc.compile()
    return nc


def _get_program():
    global _PROGRAM
    if _PROGRAM is None:
        _PROGRAM = _build_program()
    return _PROGRAM


# ---------------------------------------------------------------- host wrapper
def _prep_inputs(x, sparse_weights, offset):
    x = np.asarray(x, np.float32)
    xe = np.zeros((NBATCH, H, NK, CIN), np.float32)
    xe[:, 0::2, :67] = x[:, 0::2, 0::2]
    xe[:, 1::2, :66] = x[:, 1::2, 1::2]
    xt = np.ascontiguousarray(xe.transpose(0, 3, 1, 2)).reshape(
        NBATCH, CIN, XLEN).astype(BF16)
    sw3 = np.asarray(sparse_weights, np.float32).reshape(CIN, COUT, _NTAPS)
    # prepacked flat [128, n*COUT] in the exact SBUF layout
    wp = np.empty((128, len(_PAIRS) * COUT), np.float32)
    for s, (lo, hi) in enumerate(_PAIRS):
        wp[0:CIN, s * COUT:(s + 1) * COUT] = sw3[:, :, lo]
        wp[CIN:128, s * COUT:(s + 1) * COUT] = sw3[:, :, hi]
    # singles duplicated into both partition halves so either PE row-tile
    # can load them
    ws = np.empty((128, len(_SINGLES) * COUT), np.float32)
    for s, j in enumerate(_SINGLES):
        ws[0:CIN, s * COUT:(s + 1) * COUT] = sw3[:, :, j]
        ws[CIN:128, s * COUT:(s + 1) * COUT] = sw3[:, :, j]
    bias = np.asarray(offset, np.float32).reshape(COUT, 1)
    return xt, wp.astype(BF16), ws.astype(BF16), bias


def _make_in_maps(inputs):
    xt, wp, ws, bias = _prep_inputs(**inputs)
    return [
        {"xt": xt[c * NPER:(c + 1) * NPER], "wp": wp, "ws": ws, "bias": bias}
        for c in range(NCORES)
    ]


def kernel(x, sparse_weights, offset):
    from concourse.bass_utils import run_bass_kernel_spmd

    nc = _get_program()
    in_maps = _make_in_maps(
        {"x": x, "sparse_weights": sparse_weights, "offset": offset})
    res = run_bass_kernel_spmd(nc, in_maps, list(range(NCORES)))
    arr = np.concatenate([np.asarray(res.results[c]["out"]).astype(np.float32)
                          for c in range(NCORES)], axis=0)  # [16, 128, 201, 65]
    arr -= 1.0   # device returns elu(z) + 1
    arr = arr.transpose(0, 2, 3, 1)                         # [16, 201, 65, 128]
    full = np.zeros((NBATCH, OH, OW, COUT), np.float32)
    full[:, 0::2, 0::2, :] = arr[:, 0::2]
    full[:, 1::2, 1::2, :] = arr[:, 1::2, :64]
    full[:, _MASK == 0] = 0.0
    return full
